# revision 9
# baseline (speedup 1.0000x reference)
"""3-layer GCN + 3-layer MLP head + log_softmax on 8 Trainium2 NeuronCores.

Strategy (matches the node-partition sharding hint):
- Nodes are partitioned across the 8 cores (12500 each, padded to 12544).
- GCN is computed aggregate-first: out = relu((A @ X) @ W + b), with the
  symmetric normalization split as A = D^-1/2 (Adj+I) D^-1/2:
  dis[src] is folded into the shared feature table (X' = dis * X, applied by
  the producing core), dis[dst] is folded into the per-edge one-hot weights.
- Per layer: every core scales/produces its shard of X', an AllGather
  rebuilds the full 100352-row table in each core's HBM, then each core
  aggregates its own dst shard:
    * per-edge rows are fetched with dma_gather (4 parallel SWDGE queues,
      int16 indices over four 25088-row sub-tables of the gathered table),
    * scatter-add is a TensorE matmul: aggT[64f, 256d] += msgs[128e, 64f].T
      @ onehot[128e, 256d], where the one-hot (with dis[dst] folded in) is
      built on VectorE with a single dual-op tensor_scalar (is_equal, mult).
- The MLP head + log_softmax run node-parallel on each core's shard in a
  transposed [feat, node] layout so biases are per-partition.

All floating point is fp32; PSUM accumulation is fp32.
"""
import math
import numpy as np

N = 100000
E = 800000
D = 64
NCLS = 16
NCORE = 8
NSHR = 12500          # real nodes per core
NSH = 12544           # padded shard rows (= 98 * 128 = 49 * 256)
NTAB = NCORE * NSH    # 100352 padded table rows
BS = 256              # dst block size (stage-A moving free dim)
NB = NSH // BS        # 49 blocks per core
NBK = 4               # src-range buckets (int16 index limit)
BK = NTAB // NBK      # 25088 rows per bucket
P = 128               # chunk size (edges per chunk = SBUF partitions)
GROUPS = [list(range(4 * i, 4 * i + 4)) for i in range(12)] + [[48]]

_CACHE = {}


def _host_prep(edge_index):
    """Build per-core chunked edge metadata. Returns program structure +
    per-core arrays."""
    src = edge_index[0].astype(np.int64)
    dst = edge_index[1].astype(np.int64)
    loops = np.arange(N, dtype=np.int64)
    src = np.concatenate([src, loops])
    dst = np.concatenate([dst, loops])
    deg = np.bincount(dst, minlength=N).astype(np.float32)
    dis = 1.0 / np.sqrt(np.maximum(deg, 1.0))

    core = dst // NSHR
    dloc = dst % NSHR
    blk = dloc // BS
    ld = (dloc % BS).astype(np.float32)
    ps = (src // NSHR) * NSH + (src % NSHR)
    buck = ps // BK
    sl = (ps % BK).astype(np.int16)
    dd = dis[dst]

    # sort edges by (core, blk, buck)
    key = ((core * NB + blk) * NBK + buck).astype(np.int64)
    order = np.argsort(key, kind="stable")
    key_s = key[order]
    sl_s, ld_s, dd_s = sl[order], ld[order], dd[order]
    counts = np.bincount(key_s, minlength=NCORE * NB * NBK).reshape(NCORE, NB, NBK)
    CHB = np.ceil(counts.max(axis=0) / P).astype(np.int64)  # [NB, NBK]
    CHB = np.maximum(CHB, 1)

    # chunk stream order: group -> bucket -> block
    chunk_off = np.zeros((NB, NBK), dtype=np.int64)  # chunk index of (b,k) range
    call_ranges = []  # per group: list of (k, c0, nch)
    block_chain = [[] for _ in range(NB)]  # per block: [(k, call-local ci, global c)]
    c = 0
    for g in GROUPS:
        for k in range(NBK):
            c0 = c
            for b in g:
                chunk_off[b, k] = c
                local0 = c - c0
                for j in range(CHB[b, k]):
                    block_chain[b].append((k, local0 + j, c + j))
                c += CHB[b, k]
            call_ranges.append((k, c0, c - c0))
    totch = c
    totslots = totch * P

    # per-core slot fill
    seg_start = np.zeros(NCORE * NB * NBK + 1, dtype=np.int64)
    np.cumsum(counts.reshape(-1), out=seg_start[1:])
    idx_cores, meta_cores = [], []
    slot_base = (chunk_off * P)  # [NB, NBK] slot start (same for all cores)
    for cid in range(NCORE):
        sl_a = np.zeros(totslots, dtype=np.int16)
        ld_a = np.zeros(totslots, dtype=np.float32)
        dd_a = np.zeros(totslots, dtype=np.float32)
        seg_ids = (cid * NB * NBK) + np.arange(NB * NBK)
        starts = seg_start[seg_ids]
        cnts = counts[cid].reshape(-1)
        # destination slot for each edge of this core
        dest = np.concatenate(
            [slot_base.reshape(-1)[i] + np.arange(cnts[i]) for i in range(NB * NBK)]
        ) if cnts.sum() else np.zeros(0, dtype=np.int64)
        srcpos = np.concatenate(
            [starts[i] + np.arange(cnts[i]) for i in range(NB * NBK)]
        ) if cnts.sum() else np.zeros(0, dtype=np.int64)
        sl_a[dest] = sl_s[srcpos]
        ld_a[dest] = ld_s[srcpos]
        dd_a[dest] = dd_s[srcpos]
        # idx tile: slot j -> [j%16 (+16r), j//16], int16, replicated x8
        idx_t = np.tile(sl_a.reshape(-1, 16).T, (8, 1)).copy()
        # meta tile: [128, 2*totch]: chunk c slots c*128..: partition p
        meta_t = np.zeros((P, 2 * totch), dtype=np.float32)
        meta_t[:, 0::2] = ld_a.reshape(totch, P).T
        meta_t[:, 1::2] = dd_a.reshape(totch, P).T
        idx_cores.append(idx_t)
        meta_cores.append(meta_t)

    # per-core dis tiles [128, 98]: dis_sb[p, t] = dis[cid*NSHR + t*128+p] (1.0 pad)
    dis_cores = []
    for cid in range(NCORE):
        dpad = np.ones(NSH, dtype=np.float32)
        dpad[:NSHR] = dis[cid * NSHR : (cid + 1) * NSHR]
        dis_cores.append(dpad.reshape(NSH // P, P).T.copy())

    struct = dict(CHB=CHB, call_ranges=call_ranges, block_chain=block_chain,
                  totch=totch, totslots=totslots)
    percore = dict(idx=idx_cores, meta=meta_cores, dis=dis_cores)
    return struct, percore


def _build_program(struct, repeat=1):
    import concourse.bass as bass
    import concourse.tile as tile
    from concourse import bacc, mybir
    from contextlib import ExitStack

    totch = struct["totch"]
    call_ranges = struct["call_ranges"]
    block_chain = struct["block_chain"]
    f32 = mybir.dt.float32

    nc = bacc.Bacc("TRN2", target_bir_lowering=False, debug=False,
                   num_devices=NCORE, num_swdge_queues=NBK)

    x_in = nc.dram_tensor("x_shard", [NSH, D], f32, kind="ExternalInput")
    idx_in = nc.dram_tensor("idx16", [P, struct["totslots"] // 16], mybir.dt.int16,
                            kind="ExternalInput")
    meta_in = nc.dram_tensor("meta", [P, 2 * totch], f32, kind="ExternalInput")
    dis_in = nc.dram_tensor("dis_sb", [P, NSH // P], f32, kind="ExternalInput")
    iota_in = nc.dram_tensor("iota", [P, BS], f32, kind="ExternalInput")
    eye_in = nc.dram_tensor("eye16", [16, 16], f32, kind="ExternalInput")
    w_ins = {}
    for nm, shp in [("W1", [D, D]), ("W2", [D, D]), ("W3", [D, D]),
                    ("Wf1", [D, D]), ("Wf2", [D, D]), ("Wf3", [D, NCLS]),
                    ("b1b", [P, D]), ("b2b", [P, D]), ("b3c", [D, 1]),
                    ("bf1c", [D, 1]), ("bf2c", [D, 1]), ("bf3c", [NCLS, 1])]:
        w_ins[nm] = nc.dram_tensor(nm, shp, f32, kind="ExternalInput")
    out_dram = nc.dram_tensor("out", [NSH, NCLS], f32, kind="ExternalOutput")

    with tile.TileContext(nc) as tc, ExitStack() as ctx:
        consts = ctx.enter_context(tc.tile_pool(name="consts", bufs=1))
        msgsp = ctx.enter_context(tc.tile_pool(name="msgsp", bufs=2))
        work = ctx.enter_context(tc.tile_pool(name="work", bufs=3))
        bigs = ctx.enter_context(tc.tile_pool(name="bigs", bufs=1))
        psp = ctx.enter_context(tc.tile_pool(name="psp", bufs=3, space="PSUM"))
        dram = ctx.enter_context(tc.tile_pool(name="dram", bufs=1, space="DRAM"))

        idx_t = consts.tile([P, struct["totslots"] // 16], mybir.dt.int16)
        nc.sync.dma_start(idx_t[:], idx_in[:])
        meta_t = consts.tile([P, 2 * totch], f32)
        nc.sync.dma_start(meta_t[:], meta_in[:])
        dis_t = consts.tile([P, NSH // P], f32)
        nc.sync.dma_start(dis_t[:], dis_in[:])
        iota_t = consts.tile([P, BS], f32)
        nc.sync.dma_start(iota_t[:], iota_in[:])
        eye_t = consts.tile([16, 16], f32)
        nc.sync.dma_start(eye_t[:], eye_in[:])
        w_t = {}
        for nm in w_ins:
            w_t[nm] = consts.tile(list(w_ins[nm].shape), f32, name=f"t_{nm}")
            nc.sync.dma_start(w_t[nm][:], w_ins[nm][:])

        rg = [list(range(NCORE))]
        y3T_big = bigs.tile([D, NSH], f32, tag="bigT")

        for _rep in range(repeat):
            agin = [dram.tile([NSH, D], f32, name=f"agin{l}_{_rep}")
                    for l in range(3)]
            tabs = [dram.tile([NTAB, D], f32, addr_space="Shared",
                              name=f"tab{l}_{_rep}") for l in range(3)]
            # ---- input prep: X'0 = dis * x ----
            for t in range(NSH // P):
                xt = work.tile([P, D], f32, tag="xt")
                nc.sync.dma_start(xt[:], x_in[t * P:(t + 1) * P, :])
                xs = work.tile([P, D], f32, tag="xs")
                nc.vector.tensor_scalar_mul(xs[:], xt[:], dis_t[:, t:t + 1])
                nc.sync.dma_start(agin[0][t * P:(t + 1) * P, :], xs[:])
            _run_net(nc, tc, mybir, struct, w_t, idx_t, meta_t, dis_t, iota_t,
                     eye_t, agin, tabs, y3T_big, msgsp, work, bigs, psp,
                     out_dram, rg)

    nc.compile()
    return nc


def _run_net(nc, tc, mybir, struct, w_t, idx_t, meta_t, dis_t, iota_t, eye_t,
             agin, tabs, y3T_big, msgsp, work, bigs, psp, out_dram, rg):
    f32 = mybir.dt.float32
    totch = struct["totch"]
    call_ranges = struct["call_ranges"]
    block_chain = struct["block_chain"]
    if True:
        for l in range(3):
            nc.gpsimd.collective_compute(
                "AllGather", mybir.AluOpType.bypass, replica_groups=rg,
                ins=[agin[l][:]], outs=[tabs[l][:]],
            )
            Wl = w_t[f"W{l + 1}"]
            for gi, g in enumerate(GROUPS):
                msgs = {}
                for (k, c0, nch) in call_ranges[gi * NBK:(gi + 1) * NBK]:
                    if nch == 0:
                        continue
                    m = msgsp.tile([P, nch * D], f32, tag=f"m{k}",
                                   name=f"msgs_l{l}_g{gi}_k{k}")
                    nc.gpsimd.dma_gather(
                        out_ap=m[:].rearrange("p (c d) -> p c d", c=nch),
                        in_ap=tabs[l][k * BK:(k + 1) * BK, :],
                        idxs_ap=idx_t[:, c0 * 8:(c0 + nch) * 8],
                        num_idxs=nch * P, num_idxs_reg=nch * P,
                        elem_size=D, single_packet=False, queue_num=k,
                    )
                    msgs[k] = m
                for b in g:
                    aggT_ps = psp.tile([D, BS], f32, tag="aggT")
                    chain = block_chain[b]
                    for i, (k, ci, gc) in enumerate(chain):
                        oh = work.tile([P, BS], f32, tag="oh")
                        nc.vector.tensor_scalar(
                            out=oh[:], in0=iota_t[:],
                            scalar1=meta_t[:, 2 * gc:2 * gc + 1],
                            scalar2=meta_t[:, 2 * gc + 1:2 * gc + 2],
                            op0=mybir.AluOpType.is_equal,
                            op1=mybir.AluOpType.mult,
                        )
                        nc.tensor.matmul(
                            aggT_ps[:], lhsT=msgs[k][:, ci * D:(ci + 1) * D],
                            rhs=oh[:], start=(i == 0), stop=(i == len(chain) - 1),
                        )
                    aggT_sb = work.tile([D, BS], f32, tag="aggT_sb")
                    nc.vector.tensor_copy(aggT_sb[:], aggT_ps[:])
                    if l < 2:
                        bias_b = w_t[f"b{l + 1}b"]
                        for h in range(2):
                            y_ps = psp.tile([P, D], f32, tag="yps")
                            nc.tensor.matmul(
                                y_ps[:], lhsT=aggT_sb[:, h * P:(h + 1) * P],
                                rhs=Wl[:], start=True, stop=True,
                            )
                            t_sb = work.tile([P, D], f32, tag="tsb")
                            nc.vector.tensor_tensor(
                                out=t_sb[:], in0=y_ps[:], in1=bias_b[:],
                                op=mybir.AluOpType.add,
                            )
                            xn = work.tile([P, D], f32, tag="xn")
                            tcol = 2 * b + h
                            nc.vector.tensor_scalar(
                                out=xn[:], in0=t_sb[:],
                                scalar1=0.0, scalar2=dis_t[:, tcol:tcol + 1],
                                op0=mybir.AluOpType.max,
                                op1=mybir.AluOpType.mult,
                            )
                            r0 = b * BS + h * P
                            nc.sync.dma_start(agin[l + 1][r0:r0 + P, :], xn[:])
                    else:
                        y3_ps = psp.tile([D, BS], f32, tag="aggT")
                        nc.tensor.matmul(y3_ps[:], lhsT=w_t["W3"][:],
                                         rhs=aggT_sb[:], start=True, stop=True)
                        nc.scalar.activation(
                            y3T_big[:, b * BS:(b + 1) * BS], y3_ps[:],
                            mybir.ActivationFunctionType.Relu,
                            bias=w_t["b3c"][:, 0:1],
                        )

        # ---- head (fused per 512-node tile) ----
        TS = 512
        tiles = [(i * TS, min(TS, NSH - i * TS)) for i in range(math.ceil(NSH / TS))]
        outbuf = bigs.tile([P, (NSH // P) * NCLS], f32, bufs=1)
        for (o, n) in tiles:
            h1_ps = psp.tile([D, TS], f32, tag="aggT", name="h1_ps")
            nc.tensor.matmul(h1_ps[:, :n], lhsT=w_t["Wf1"][:],
                             rhs=y3T_big[:, o:o + n], start=True, stop=True)
            h1_sb = work.tile([D, TS], f32, tag="h1sb")
            nc.scalar.activation(h1_sb[:, :n], h1_ps[:, :n],
                                 mybir.ActivationFunctionType.Relu,
                                 bias=w_t["bf1c"][:, 0:1])
            h2_ps = psp.tile([D, TS], f32, tag="aggT", name="h2_ps")
            nc.tensor.matmul(h2_ps[:, :n], lhsT=w_t["Wf2"][:],
                             rhs=h1_sb[:, :n], start=True, stop=True)
            h2_sb = work.tile([D, TS], f32, tag="h2sb")
            nc.scalar.activation(h2_sb[:, :n], h2_ps[:, :n],
                                 mybir.ActivationFunctionType.Relu,
                                 bias=w_t["bf2c"][:, 0:1])
            h3_ps = psp.tile([NCLS, TS], f32, tag="yps")
            nc.tensor.matmul(h3_ps[:, :n], lhsT=w_t["Wf3"][:],
                             rhs=h2_sb[:, :n], start=True, stop=True)
            h3_sb = work.tile([NCLS, TS], f32, tag="h3sb")
            nc.vector.tensor_scalar(
                out=h3_sb[:, :n], in0=h3_ps[:, :n],
                scalar1=w_t["bf3c"][:, 0:1], scalar2=None,
                op0=mybir.AluOpType.add, op1=mybir.AluOpType.bypass,
            )
            for j in range(n // P):
                t = o // P + j
                tr_ps = psp.tile([P, NCLS], f32, tag="yps", name="tr_ps")
                nc.tensor.transpose(tr_ps[:], h3_sb[:, j * P:(j + 1) * P], eye_t[:])
                mx = work.tile([P, 1], f32, tag="mx")
                nc.vector.reduce_max(mx[:], tr_ps[:], axis=mybir.AxisListType.X)
                nm = work.tile([P, 1], f32, tag="nm")
                nc.vector.tensor_scalar_mul(nm[:], mx[:], -1.0)
                ex = work.tile([P, NCLS], f32, tag="ex")
                nc.scalar.activation(ex[:], tr_ps[:],
                                     mybir.ActivationFunctionType.Exp,
                                     bias=nm[:, 0:1])
                sm = work.tile([P, 1], f32, tag="sm")
                nc.vector.reduce_sum(sm[:], ex[:], axis=mybir.AxisListType.X)
                ls = work.tile([P, 1], f32, tag="ls")
                nc.scalar.activation(ls[:], sm[:],
                                     mybir.ActivationFunctionType.Ln)
                nc.vector.tensor_scalar(
                    out=outbuf[:, t * NCLS:(t + 1) * NCLS], in0=tr_ps[:],
                    scalar1=mx[:, 0:1], scalar2=ls[:, 0:1],
                    op0=mybir.AluOpType.subtract, op1=mybir.AluOpType.subtract,
                )
        nc.sync.dma_start(
            out_dram[:].rearrange("(t p) f -> p t f", p=P),
            outbuf[:].rearrange("p (t f) -> p t f", f=NCLS),
        )


def _get_program_and_prep(edge_index):
    key = hash(edge_index.tobytes())
    if key not in _CACHE:
        struct, percore = _host_prep(edge_index)
        nc = _build_program(struct)
        _CACHE[key] = (nc, percore)
    return _CACHE[key]


def kernel(**inputs):
    x = np.ascontiguousarray(np.asarray(inputs["x"], dtype=np.float32))
    edge_index = np.ascontiguousarray(np.asarray(inputs["edge_index"]))
    nc, percore = _get_program_and_prep(edge_index)

    iota = np.tile(np.arange(BS, dtype=np.float32)[None, :], (P, 1))
    eye16 = np.eye(16, dtype=np.float32)
    ones_col = np.ones((P, 1), dtype=np.float32)
    common = {
        "iota": iota, "eye16": eye16,
        "W1": np.asarray(inputs["W1"], np.float32),
        "W2": np.asarray(inputs["W2"], np.float32),
        "W3": np.asarray(inputs["W3"], np.float32),
        "Wf1": np.asarray(inputs["Wf1"], np.float32),
        "Wf2": np.asarray(inputs["Wf2"], np.float32),
        "Wf3": np.asarray(inputs["Wf3"], np.float32),
        "b1b": ones_col * np.asarray(inputs["b1"], np.float32)[None, :],
        "b2b": ones_col * np.asarray(inputs["b2"], np.float32)[None, :],
        "b3c": np.asarray(inputs["b3"], np.float32)[:, None],
        "bf1c": np.asarray(inputs["bf1"], np.float32)[:, None],
        "bf2c": np.asarray(inputs["bf2"], np.float32)[:, None],
        "bf3c": np.asarray(inputs["bf3"], np.float32)[:, None],
    }
    in_maps = []
    for cid in range(NCORE):
        xsh = np.zeros((NSH, D), dtype=np.float32)
        xsh[:NSHR] = x[cid * NSHR:(cid + 1) * NSHR]
        m = dict(common)
        m.update({
            "x_shard": xsh,
            "idx16": percore["idx"][cid],
            "meta": percore["meta"][cid],
            "dis_sb": percore["dis"][cid],
        })
        in_maps.append(m)

    from concourse.bass_utils import run_bass_kernel_spmd
    res = run_bass_kernel_spmd(nc, in_maps, core_ids=list(range(NCORE)))
    out = np.empty((N, NCLS), dtype=np.float32)
    for cid in range(NCORE):
        out[cid * NSHR:(cid + 1) * NSHR] = res.results[cid]["out"][:NSHR]
    return out


# revision 23
# speedup vs baseline: 1.6490x; 1.6490x over previous
"""3-layer GCN + 3-layer MLP head + log_softmax on 8 Trainium2 NeuronCores.

Strategy (matches the node-partition sharding hint):
- Nodes are partitioned across the 8 cores (12500 each, padded to 12544).
- GCN is computed aggregate-first: out = relu((A @ X) @ W + b), with the
  symmetric normalization split as A = D^-1/2 (Adj+I) D^-1/2:
  dis[src] is folded into the shared feature table (X' = dis * X, applied by
  the producing core), dis[dst] is folded into the per-edge one-hot weights.
- Per layer: every core scales/produces its shard of X', an AllGather
  rebuilds the full 100352-row table in each core's HBM, then each core
  aggregates its own dst shard:
    * per-edge rows are fetched with dma_gather (4 parallel SWDGE queues,
      int16 indices over four 25088-row sub-tables of the gathered table),
    * scatter-add is a TensorE matmul: aggT[64f, 256d] += msgs[128e, 64f].T
      @ onehot[128e, 256d], where the one-hot (with dis[dst] folded in) is
      built on VectorE with a single dual-op tensor_scalar (is_equal, mult).
- The MLP head + log_softmax run node-parallel on each core's shard in a
  transposed [feat, node] layout so biases are per-partition.

All floating point is fp32; PSUM accumulation is fp32.
"""
import math
import numpy as np

N = 100000
E = 800000
D = 64
NCLS = 16
NCORE = 8
NSHR = 12500          # real nodes per core
NSH = 12544           # padded shard rows (= 98 * 128 = 49 * 256)
NTAB = NCORE * NSH    # 100352 padded table rows
BS = 256              # dst block size (stage-A moving free dim)
NB = NSH // BS        # 49 blocks per core
NBK = 4               # src-range buckets (int16 index limit)
BK = NTAB // NBK      # 25088 rows per bucket
P = 128               # chunk size (edges per chunk = SBUF partitions)
GROUPS = [list(range(4 * i, 4 * i + 4)) for i in range(12)] + [[48]]

_CACHE = {}


def _host_prep(edge_index):
    """Build per-core chunked edge metadata. Returns program structure +
    per-core arrays."""
    src = edge_index[0].astype(np.int64)
    dst = edge_index[1].astype(np.int64)
    # degree includes the self-loop; the self-loop term itself is added on
    # device via an identity matmul (no gather), so it is NOT in the edge list
    deg = (np.bincount(dst, minlength=N) + 1).astype(np.float32)
    dis = 1.0 / np.sqrt(np.maximum(deg, 1.0))

    core = dst // NSHR
    dloc = dst % NSHR
    blk = dloc // BS
    ld = (dloc % BS).astype(np.float32)
    ps = (src // NSHR) * NSH + (src % NSHR)
    buck = ps // BK
    sl = (ps % BK).astype(np.int16)
    dd = dis[dst]

    # sort edges by (core, blk, buck)
    key = ((core * NB + blk) * NBK + buck).astype(np.int64)
    order = np.argsort(key, kind="stable")
    key_s = key[order]
    sl_s, ld_s, dd_s = sl[order], ld[order], dd[order]
    counts = np.bincount(key_s, minlength=NCORE * NB * NBK).reshape(NCORE, NB, NBK)
    CHB = np.ceil(counts.max(axis=0) / P).astype(np.int64)  # [NB, NBK]
    CHB = np.maximum(CHB, 1)

    # chunk stream order: group -> bucket -> block
    chunk_off = np.zeros((NB, NBK), dtype=np.int64)  # chunk index of (b,k) range
    call_ranges = []  # per group: list of (k, c0, nch)
    block_chain = [[] for _ in range(NB)]  # per block: [(k, call-local ci, global c)]
    c = 0
    for g in GROUPS:
        for k in range(NBK):
            c0 = c
            for b in g:
                chunk_off[b, k] = c
                local0 = c - c0
                for j in range(CHB[b, k]):
                    block_chain[b].append((k, local0 + j, c + j))
                c += CHB[b, k]
            call_ranges.append((k, c0, c - c0))
    totch = c
    totslots = totch * P

    # per-core slot fill
    seg_start = np.zeros(NCORE * NB * NBK + 1, dtype=np.int64)
    np.cumsum(counts.reshape(-1), out=seg_start[1:])
    idx_cores, meta_cores = [], []
    slot_base = (chunk_off * P)  # [NB, NBK] slot start (same for all cores)
    for cid in range(NCORE):
        sl_a = np.zeros(totslots, dtype=np.int16)
        ld_a = np.zeros(totslots, dtype=np.float32)
        dd_a = np.zeros(totslots, dtype=np.float32)
        seg_ids = (cid * NB * NBK) + np.arange(NB * NBK)
        starts = seg_start[seg_ids]
        cnts = counts[cid].reshape(-1)
        # destination slot for each edge of this core
        dest = np.concatenate(
            [slot_base.reshape(-1)[i] + np.arange(cnts[i]) for i in range(NB * NBK)]
        ) if cnts.sum() else np.zeros(0, dtype=np.int64)
        srcpos = np.concatenate(
            [starts[i] + np.arange(cnts[i]) for i in range(NB * NBK)]
        ) if cnts.sum() else np.zeros(0, dtype=np.int64)
        sl_a[dest] = sl_s[srcpos]
        ld_a[dest] = ld_s[srcpos]
        dd_a[dest] = dd_s[srcpos]
        # idx tile: slot j -> [j%16 (+16r), j//16], int16, replicated x8
        idx_t = np.tile(sl_a.reshape(-1, 16).T, (8, 1)).copy()
        # meta tile: [128, 2*totch]: chunk c slots c*128..: partition p
        meta_t = np.zeros((P, 2 * totch), dtype=np.float32)
        meta_t[:, 0::2] = ld_a.reshape(totch, P).T
        meta_t[:, 1::2] = dd_a.reshape(totch, P).T
        idx_cores.append(idx_t)
        meta_cores.append(meta_t)

    # per-core dis tiles [128, 98]: dis_sb[p, t] = dis[cid*NSHR + t*128+p] (1.0 pad)
    dis_cores = []
    for cid in range(NCORE):
        dpad = np.ones(NSH, dtype=np.float32)
        dpad[:NSHR] = dis[cid * NSHR : (cid + 1) * NSHR]
        dis_cores.append(dpad.reshape(NSH // P, P).T.copy())

    struct = dict(CHB=CHB, call_ranges=call_ranges, block_chain=block_chain,
                  totch=totch, totslots=totslots)
    percore = dict(idx=idx_cores, meta=meta_cores, dis=dis_cores)
    return struct, percore


def _build_program(struct, repeat=1, parts="full"):
    import concourse.bass as bass
    import concourse.tile as tile
    from concourse import bacc, mybir
    from contextlib import ExitStack

    totch = struct["totch"]
    call_ranges = struct["call_ranges"]
    block_chain = struct["block_chain"]
    f32 = mybir.dt.float32

    nc = bacc.Bacc("TRN2", target_bir_lowering=False, debug=False,
                   num_devices=NCORE, num_swdge_queues=NBK)

    x_in = nc.dram_tensor("x_shard", [NSH, D], f32, kind="ExternalInput")
    idx_in = nc.dram_tensor("idx16", [P, struct["totslots"] // 16], mybir.dt.int16,
                            kind="ExternalInput")
    meta_in = nc.dram_tensor("meta", [P, 2 * totch], f32, kind="ExternalInput")
    dis_in = nc.dram_tensor("dis_sb", [P, NSH // P], f32, kind="ExternalInput")
    iota_in = nc.dram_tensor("iota", [P, BS], f32, kind="ExternalInput")
    eye_in = nc.dram_tensor("eye16", [16, 16], f32, kind="ExternalInput")
    eye128_in = nc.dram_tensor("eye128", [P, P], f32, kind="ExternalInput")
    w_ins = {}
    for nm, shp in [("W1", [D, D]), ("W2", [D, D]), ("W3", [D, D]),
                    ("Wf1", [D, D]), ("Wf2", [D, D]), ("Wf3", [D, NCLS]),
                    ("b1b", [P, D]), ("b2b", [P, D]), ("b3c", [D, 1]),
                    ("bf1c", [D, 1]), ("bf2c", [D, 1]), ("bf3c", [NCLS, 1])]:
        w_ins[nm] = nc.dram_tensor(nm, shp, f32, kind="ExternalInput")
    out_dram = nc.dram_tensor("out", [NSH, NCLS], f32, kind="ExternalOutput")

    with tile.TileContext(nc) as tc, ExitStack() as ctx:
        consts = ctx.enter_context(tc.tile_pool(name="consts", bufs=1))
        msgsp = ctx.enter_context(tc.tile_pool(name="msgsp", bufs=2))
        work = ctx.enter_context(tc.tile_pool(name="work", bufs=3))
        bigs = ctx.enter_context(tc.tile_pool(name="bigs", bufs=1))
        psp = ctx.enter_context(tc.tile_pool(name="psp", bufs=3, space="PSUM"))
        dram = ctx.enter_context(tc.tile_pool(name="dram", bufs=1, space="DRAM"))

        idx_t = consts.tile([P, struct["totslots"] // 16], mybir.dt.int16)
        nc.sync.dma_start(idx_t[:], idx_in[:])
        meta_t = consts.tile([P, 2 * totch], f32)
        nc.sync.dma_start(meta_t[:], meta_in[:])
        dis_t = consts.tile([P, NSH // P], f32)
        nc.sync.dma_start(dis_t[:], dis_in[:])
        iota_t = consts.tile([P, BS], f32)
        nc.sync.dma_start(iota_t[:], iota_in[:])
        eye_t = consts.tile([16, 16], f32)
        nc.sync.dma_start(eye_t[:], eye_in[:])
        eye128_t = consts.tile([P, P], f32)
        nc.sync.dma_start(eye128_t[:], eye128_in[:])
        w_t = {}
        for nm in w_ins:
            w_t[nm] = consts.tile(list(w_ins[nm].shape), f32, name=f"t_{nm}")
            nc.sync.dma_start(w_t[nm][:], w_ins[nm][:])

        rg = [list(range(NCORE))]
        y3T_big = bigs.tile([D, NSH], f32, tag="bigT")
        xkeep = bigs.tile([P, (NSH // P) * D], f32, bufs=1)

        for _rep in range(repeat):
            agin = [dram.tile([NSH, D], f32, name=f"agin{l}_{_rep}")
                    for l in range(3)]
            tabs = [dram.tile([NTAB, D], f32, addr_space="Shared",
                              name=f"tab{l}_{_rep}") for l in range(3)]
            # ---- input prep: X'0 = dis * x ----
            for t in range(NSH // P):
                xt = work.tile([P, D], f32, tag="xt")
                nc.sync.dma_start(xt[:], x_in[t * P:(t + 1) * P, :])
                nc.vector.tensor_scalar_mul(
                    xkeep[:, t * D:(t + 1) * D], xt[:], dis_t[:, t:t + 1])
                nc.sync.dma_start(agin[0][t * P:(t + 1) * P, :],
                                  xkeep[:, t * D:(t + 1) * D])
            _run_net(nc, tc, mybir, struct, w_t, idx_t, meta_t, dis_t, iota_t,
                     eye_t, eye128_t, xkeep, agin, tabs, y3T_big, msgsp, work,
                     bigs, psp, out_dram, rg, parts)

    nc.compile()
    return nc


def _run_net(nc, tc, mybir, struct, w_t, idx_t, meta_t, dis_t, iota_t, eye_t,
             eye128_t, xkeep, agin, tabs, y3T_big, msgsp, work, bigs, psp,
             out_dram, rg, parts="full"):
    f32 = mybir.dt.float32
    totch = struct["totch"]
    call_ranges = struct["call_ranges"]
    block_chain = struct["block_chain"]
    if True:
        for l in range(3):
            nc.gpsimd.collective_compute(
                "AllGather", mybir.AluOpType.bypass, replica_groups=rg,
                ins=[agin[l][:]], outs=[tabs[l][:]],
            )
            Wl = w_t[f"W{l + 1}"]
            for gi, g in enumerate(GROUPS):
                msgs = {}
                for (k, c0, nch) in call_ranges[gi * NBK:(gi + 1) * NBK]:
                    if nch == 0:
                        continue
                    m = msgsp.tile([P, nch * D], f32, tag=f"m{k}",
                                   name=f"msgs_l{l}_g{gi}_k{k}")
                    nc.gpsimd.dma_gather(
                        out_ap=m[:].rearrange("p (c d) -> p c d", c=nch),
                        in_ap=tabs[l][k * BK:(k + 1) * BK, :],
                        idxs_ap=idx_t[:, c0 * 8:(c0 + nch) * 8],
                        num_idxs=nch * P, num_idxs_reg=nch * P,
                        elem_size=D, single_packet=False, queue_num=k,
                    )
                    msgs[k] = m
                if parts == "g":
                    continue
                for b in g:
                    aggT_ps = psp.tile([D, BS], f32, tag="aggT")
                    chain = block_chain[b]
                    for i, (k, ci, gc) in enumerate(chain):
                        oh = work.tile([P, BS], f32, tag="oh")
                        nc.vector.tensor_scalar(
                            out=oh[:], in0=iota_t[:],
                            scalar1=meta_t[:, 2 * gc:2 * gc + 1],
                            scalar2=meta_t[:, 2 * gc + 1:2 * gc + 2],
                            op0=mybir.AluOpType.is_equal,
                            op1=mybir.AluOpType.mult,
                        )
                        if parts == "gd":
                            continue
                        nc.tensor.matmul(
                            aggT_ps[:], lhsT=msgs[k][:, ci * D:(ci + 1) * D],
                            rhs=oh[:], start=(i == 0), stop=False,
                        )
                    if parts == "gd":
                        continue
                    # self-loop term: aggT[:, h*128+d] += dis[d] * x'[d, :]
                    # via a diagonal-matrix matmul (diag = eye128 * dis_block)
                    for h in range(2):
                        tcol = 2 * b + h
                        diag = work.tile([P, P], f32, tag="diag")
                        nc.vector.tensor_scalar_mul(
                            diag[:], eye128_t[:], dis_t[:, tcol:tcol + 1])
                        nc.tensor.matmul(
                            aggT_ps[:, h * P:(h + 1) * P],
                            lhsT=xkeep[:, tcol * D:(tcol + 1) * D],
                            rhs=diag[:], start=False, stop=(h == 1),
                        )
                    if parts == "gdm":
                        continue
                    aggT_sb = work.tile([D, BS], f32, tag="aggT_sb")
                    nc.vector.tensor_copy(aggT_sb[:], aggT_ps[:])
                    if l < 2:
                        bias_b = w_t[f"b{l + 1}b"]
                        for h in range(2):
                            y_ps = psp.tile([P, D], f32, tag="yps")
                            nc.tensor.matmul(
                                y_ps[:], lhsT=aggT_sb[:, h * P:(h + 1) * P],
                                rhs=Wl[:], start=True, stop=True,
                            )
                            t_sb = work.tile([P, D], f32, tag="tsb")
                            nc.vector.tensor_tensor(
                                out=t_sb[:], in0=y_ps[:], in1=bias_b[:],
                                op=mybir.AluOpType.add,
                            )
                            tcol = 2 * b + h
                            nc.vector.tensor_scalar(
                                out=xkeep[:, tcol * D:(tcol + 1) * D],
                                in0=t_sb[:],
                                scalar1=0.0, scalar2=dis_t[:, tcol:tcol + 1],
                                op0=mybir.AluOpType.max,
                                op1=mybir.AluOpType.mult,
                            )
                            r0 = b * BS + h * P
                            nc.sync.dma_start(agin[l + 1][r0:r0 + P, :],
                                              xkeep[:, tcol * D:(tcol + 1) * D])
                    else:
                        y3_ps = psp.tile([D, BS], f32, tag="aggT")
                        nc.tensor.matmul(y3_ps[:], lhsT=w_t["W3"][:],
                                         rhs=aggT_sb[:], start=True, stop=True)
                        nc.scalar.activation(
                            y3T_big[:, b * BS:(b + 1) * BS], y3_ps[:],
                            mybir.ActivationFunctionType.Relu,
                            bias=w_t["b3c"][:, 0:1],
                        )

        # ---- head (fused per 512-node tile) ----
        TS = 512
        tiles = [(i * TS, min(TS, NSH - i * TS)) for i in range(math.ceil(NSH / TS))]
        outbuf = bigs.tile([P, (NSH // P) * NCLS], f32, bufs=1)
        if parts != "full":
            nc.vector.memset(outbuf[:], 0.0)
            nc.sync.dma_start(
                out_dram[:].rearrange("(t p) f -> p t f", p=P),
                outbuf[:].rearrange("p (t f) -> p t f", f=NCLS),
            )
            return
        for (o, n) in tiles:
            h1_ps = psp.tile([D, TS], f32, tag="aggT", name="h1_ps")
            nc.tensor.matmul(h1_ps[:, :n], lhsT=w_t["Wf1"][:],
                             rhs=y3T_big[:, o:o + n], start=True, stop=True)
            h1_sb = work.tile([D, TS], f32, tag="h1sb")
            nc.scalar.activation(h1_sb[:, :n], h1_ps[:, :n],
                                 mybir.ActivationFunctionType.Relu,
                                 bias=w_t["bf1c"][:, 0:1])
            h2_ps = psp.tile([D, TS], f32, tag="aggT", name="h2_ps")
            nc.tensor.matmul(h2_ps[:, :n], lhsT=w_t["Wf2"][:],
                             rhs=h1_sb[:, :n], start=True, stop=True)
            h2_sb = work.tile([D, TS], f32, tag="h2sb")
            nc.scalar.activation(h2_sb[:, :n], h2_ps[:, :n],
                                 mybir.ActivationFunctionType.Relu,
                                 bias=w_t["bf2c"][:, 0:1])
            h3_ps = psp.tile([NCLS, TS], f32, tag="yps")
            nc.tensor.matmul(h3_ps[:, :n], lhsT=w_t["Wf3"][:],
                             rhs=h2_sb[:, :n], start=True, stop=True)
            h3_sb = work.tile([NCLS, TS], f32, tag="h3sb")
            nc.vector.tensor_scalar(
                out=h3_sb[:, :n], in0=h3_ps[:, :n],
                scalar1=w_t["bf3c"][:, 0:1], scalar2=None,
                op0=mybir.AluOpType.add, op1=mybir.AluOpType.bypass,
            )
            for j in range(n // P):
                t = o // P + j
                tr_ps = psp.tile([P, NCLS], f32, tag="yps", name="tr_ps")
                nc.tensor.transpose(tr_ps[:], h3_sb[:, j * P:(j + 1) * P], eye_t[:])
                mx = work.tile([P, 1], f32, tag="mx")
                nc.vector.reduce_max(mx[:], tr_ps[:], axis=mybir.AxisListType.X)
                nm = work.tile([P, 1], f32, tag="nm")
                nc.vector.tensor_scalar_mul(nm[:], mx[:], -1.0)
                ex = work.tile([P, NCLS], f32, tag="ex")
                nc.scalar.activation(ex[:], tr_ps[:],
                                     mybir.ActivationFunctionType.Exp,
                                     bias=nm[:, 0:1])
                sm = work.tile([P, 1], f32, tag="sm")
                nc.vector.reduce_sum(sm[:], ex[:], axis=mybir.AxisListType.X)
                ls = work.tile([P, 1], f32, tag="ls")
                nc.scalar.activation(ls[:], sm[:],
                                     mybir.ActivationFunctionType.Ln)
                nc.vector.tensor_scalar(
                    out=outbuf[:, t * NCLS:(t + 1) * NCLS], in0=tr_ps[:],
                    scalar1=mx[:, 0:1], scalar2=ls[:, 0:1],
                    op0=mybir.AluOpType.subtract, op1=mybir.AluOpType.subtract,
                )
        nc.sync.dma_start(
            out_dram[:].rearrange("(t p) f -> p t f", p=P),
            outbuf[:].rearrange("p (t f) -> p t f", f=NCLS),
        )


def _get_program_and_prep(edge_index):
    key = hash(edge_index.tobytes())
    if key not in _CACHE:
        struct, percore = _host_prep(edge_index)
        nc = _build_program(struct)
        _CACHE[key] = (nc, percore)
    return _CACHE[key]


def kernel(**inputs):
    x = np.ascontiguousarray(np.asarray(inputs["x"], dtype=np.float32))
    edge_index = np.ascontiguousarray(np.asarray(inputs["edge_index"]))
    nc, percore = _get_program_and_prep(edge_index)

    iota = np.tile(np.arange(BS, dtype=np.float32)[None, :], (P, 1))
    eye16 = np.eye(16, dtype=np.float32)
    ones_col = np.ones((P, 1), dtype=np.float32)
    common = {
        "iota": iota, "eye16": eye16, "eye128": np.eye(P, dtype=np.float32),
        "W1": np.asarray(inputs["W1"], np.float32),
        "W2": np.asarray(inputs["W2"], np.float32),
        "W3": np.asarray(inputs["W3"], np.float32),
        "Wf1": np.asarray(inputs["Wf1"], np.float32),
        "Wf2": np.asarray(inputs["Wf2"], np.float32),
        "Wf3": np.asarray(inputs["Wf3"], np.float32),
        "b1b": ones_col * np.asarray(inputs["b1"], np.float32)[None, :],
        "b2b": ones_col * np.asarray(inputs["b2"], np.float32)[None, :],
        "b3c": np.asarray(inputs["b3"], np.float32)[:, None],
        "bf1c": np.asarray(inputs["bf1"], np.float32)[:, None],
        "bf2c": np.asarray(inputs["bf2"], np.float32)[:, None],
        "bf3c": np.asarray(inputs["bf3"], np.float32)[:, None],
    }
    in_maps = []
    for cid in range(NCORE):
        xsh = np.zeros((NSH, D), dtype=np.float32)
        xsh[:NSHR] = x[cid * NSHR:(cid + 1) * NSHR]
        m = dict(common)
        m.update({
            "x_shard": xsh,
            "idx16": percore["idx"][cid],
            "meta": percore["meta"][cid],
            "dis_sb": percore["dis"][cid],
        })
        in_maps.append(m)

    from concourse.bass_utils import run_bass_kernel_spmd
    res = run_bass_kernel_spmd(nc, in_maps, core_ids=list(range(NCORE)))
    out = np.empty((N, NCLS), dtype=np.float32)
    for cid in range(NCORE):
        out[cid * NSHR:(cid + 1) * NSHR] = res.results[cid]["out"][:NSHR]
    return out
